# revision 10
# baseline (speedup 1.0000x reference)
"""Trainium2 Bass kernel for a 2-layer LSTM (B=512, T=1024, D=128, H=256, OUT=1).

Strategy: data-parallel over batch (8 cores x 64 rows). Each core runs the full
T=1024 recurrence on its batch shard. On-chip layout is "transposed": partition
dim = feature chunk (128 wide), free dim = 64*chunk_idx + batch, so h-state
tiles are directly the moving (rhs) operand of the recurrent matmuls.

v4 design (per layer, per step):
- ONE PSUM bank [128, 512] holds all 8 gate chunks in order [i i f f o o g g];
  g-gate weight columns are pre-scaled x2 on host so a SINGLE 512-wide sigmoid
  yields sigmoid for i,f,o and s = sigmoid(2*ghat) for g, using the identity
  tanh(x) = 2*sigmoid(2x) - 1.
- Cell state is stored offset+scaled: m = c/2 + 1/2 (in [0,1]), parked in the
  SAME ping-pong tile as the sigmoid output, right after the g region:
  tile layout [i | f | o | s | m] = [128, 640]. Then
    fcig = (in0 - 0.5) * in1   with in0 = [s|m], in1 = [i|f]   (one 256-wide
                               scalar_tensor_tensor: = [i*(s-.5) | f*(m-.5)])
    m_new = (fc + 0.5) + ig    (one 128-wide scalar_tensor_tensor)
    tanh(c) = Tanh(2*m - 1)    (free scale/bias on the ACT instruction)
    h = o * tanh(c)
  This is 3 DVE ops + 2 ACT ops per layer per step (v1: 4-5 DVE + 3 ACT).
- Layer 2 runs LAGGED by one pipeline iteration: its sigmoid/tanh occupy the
  ACT-engine idle slots of the L1 recurrence cycle instead of delaying it.
  Engine-queue orders are pinned with explicit deps (the Tile scheduler
  otherwise reorders ACT/DVE and puts L2 ops on the L1 critical cycle).
- A few always-ready dummy matmuls pad the PE queue right before each rec
  group: they absorb the ~190ns post-idle PE pipeline restart that otherwise
  lands on the critical path, and keep PE activity above the HAM clock-gate
  threshold (otherwise the PE oscillates between 1.2 and 2.4 GHz).
"""

import numpy as np
import ml_dtypes

B, T, D = 512, 1024, 128
H = 256
NCORES = 8
BL = B // NCORES  # 64 batch rows per core
XBLK = 16  # timesteps per x DMA block
# on-chip chunk order [i0 i1 f0 f1 o0 o1 g0 g1]; original order f i g o.
PERM = [2, 3, 0, 1, 6, 7, 4, 5]
G_POS = (6, 7)  # on-chip chunk positions holding the g gate (weights x2)
NDUMMY = 7  # PE warm-keeper matmuls per step

_F16 = np.float16


def _build(t_steps, with_b1, with_b2):
    import concourse.bass as bass  # noqa: F401
    from concourse.tile import add_dep_helper
    import concourse.mybir as mybir
    import concourse.tile as tile
    from concourse import bacc

    dt = mybir.dt
    AF = mybir.ActivationFunctionType
    ALU = mybir.AluOpType
    nblk = (t_steps + XBLK - 1) // XBLK
    T_ = t_steps

    nc = bacc.Bacc("TRN2", target_bir_lowering=False, debug=False, num_devices=NCORES)
    x_in = nc.declare_dram_parameter(
        "x", [nblk, 128, XBLK, BL], dt.float16, isOutput=False
    )
    w1_in = nc.declare_dram_parameter("w1", [128, 3 * 8 * 128], dt.float16, isOutput=False)
    w2_in = nc.declare_dram_parameter("w2", [128, 4 * 8 * 128], dt.float16, isOutput=False)
    if with_b1:
        b1_in = nc.declare_dram_parameter("b1", [8, 128], dt.float16, isOutput=False)
    if with_b2:
        b2_in = nc.declare_dram_parameter("b2", [8, 128], dt.float16, isOutput=False)
    if with_b1 or with_b2:
        ind_in = nc.declare_dram_parameter("ind", [8, 512], dt.float16, isOutput=False)
    y_out = nc.declare_dram_parameter("y", [128, 128], dt.float32, isOutput=True)

    with tile.TileContext(nc) as tc:
        with (
            tc.tile_pool(name="singles", bufs=1) as singles,
            tc.tile_pool(name="temps", bufs=8) as temps,
            tc.tile_pool(name="psum", bufs=1, space="PSUM") as psum,
        ):
            w1 = singles.tile([128, 3 * 8 * 128], dt.float16)
            w2 = singles.tile([128, 4 * 8 * 128], dt.float16)
            nc.sync.dma_start(out=w1, in_=w1_in[:])
            nc.sync.dma_start(out=w2, in_=w2_in[:])
            if with_b1:
                b1s = singles.tile([8, 128], dt.float16)
                nc.sync.dma_start(out=b1s, in_=b1_in[:])
            if with_b2:
                b2s = singles.tile([8, 128], dt.float16)
                nc.sync.dma_start(out=b2s, in_=b2_in[:])
            if with_b1 or with_b2:
                ind = singles.tile([8, 512], dt.float16)
                nc.sync.dma_start(out=ind, in_=ind_in[:])

            xr = [
                singles.tile([128, XBLK * BL], dt.float16, name=f"xr{i}")
                for i in range(3)
            ]
            h1r = [singles.tile([128, 128], dt.float16, name=f"h1r{i}") for i in range(2)]
            h2r = [singles.tile([128, 128], dt.float16, name=f"h2r{i}") for i in range(2)]
            # ping-pong [i|f|o|s|m] tiles per layer; m region [512:640]
            fg1 = [singles.tile([128, 640], dt.float16, name=f"fg1_{i}") for i in range(2)]
            fg2 = [singles.tile([128, 640], dt.float16, name=f"fg2_{i}") for i in range(2)]
            out_sb = singles.tile([128, 128], dt.float32)
            for tl in (h1r[0], h1r[1], h2r[0], h2r[1]):
                nc.gpsimd.memset(tl, 0.0)
            # c(0) = 0
            nc.gpsimd.memset(fg1[0][:, 512:640], 0.0)
            nc.gpsimd.memset(fg2[0][:, 512:640], 0.0)

            # gb1 4-deep: xproj(t+2) prefetch must not WAR-block on sig1(t)
            gb1 = [psum.tile([128, 512], dt.float32, name=f"gb1_{i}") for i in range(4)]
            gb2 = [psum.tile([128, 512], dt.float32, name=f"gb2_{i}") for i in range(2)]
            dmy = psum.tile([128, 64], dt.float32, name="dmy")

            nc.sync.dma_start(out=xr[0], in_=x_in[0])

            mm = nc.tensor.matmul
            last = {"act": None, "dve": None}

            def pin(kind, inst):
                if last[kind] is not None:
                    add_dep_helper(inst.ins, last[kind].ins, reason=f"{kind} order")
                last[kind] = inst
                return inst

            def w1_tile(k, j):
                i = (k * 8 + j) * 128
                return w1[:, i : i + 128]

            def w2_tile(k, j):
                i = (k * 8 + j) * 128
                return w2[:, i : i + 128]

            def xs_of(t):
                blk = t // XBLK
                tt = t % XBLK
                return xr[blk % 3][:, tt * BL : (tt + 1) * BL]

            def l1_mm_xproj(t):
                p = t % 4
                blk = t // XBLK
                tt = t % XBLK
                if tt == 0 and blk + 1 < nblk:
                    nc.sync.dma_start(out=xr[(blk + 1) % 3], in_=x_in[blk + 1])
                xs = xs_of(t)
                for j in range(8):
                    mm(gb1[p][:, 64 * j : 64 * j + 64], w1_tile(0, j), xs,
                       start=(j == 0), stop=False, skip_group_check=True)
                if with_b1:
                    mm(gb1[p][:, :], b1s, ind, start=False, stop=False,
                       skip_group_check=True)

            def l1_mm_rec(t):
                p = t % 4
                h1_prev = h1r[(t + 1) % 2]
                for k in (1, 2):
                    hk = h1_prev[:, 64 * (k - 1) : 64 * k]
                    for j in range(8):
                        mm(gb1[p][:, 64 * j : 64 * j + 64], w1_tile(k, j), hk,
                           start=False, stop=(k == 2 and j == 7),
                           skip_group_check=True)

            def l2_mm_h1(t):
                # group leader (start=True): h1-dependent half
                p = t % 2
                h1_cur = h1r[t % 2]
                for k in (0, 1):
                    hk = h1_cur[:, 64 * k : 64 * (k + 1)]
                    for j in range(8):
                        mm(gb2[p][:, 64 * j : 64 * j + 64], w2_tile(k, j), hk,
                           start=(k == 0 and j == 0), stop=False,
                           skip_group_check=True)
                if with_b2:
                    mm(gb2[p][:, :], b2s, ind, start=False, stop=False,
                       skip_group_check=True)

            def l2_mm_h2(t):
                # group tail (stop=True): h2-dependent half, emitted one
                # iteration later (h2(t-1) materializes late; this keeps its
                # wait off the PE queue positions that gate the L1 cycle)
                p = t % 2
                h2_prev = h2r[(t + 1) % 2]
                for k in (2, 3):
                    hk = h2_prev[:, 64 * (k - 2) : 64 * (k - 1)]
                    for j in range(8):
                        mm(gb2[p][:, 64 * j : 64 * j + 64], w2_tile(k, j), hk,
                           start=False, stop=(k == 3 and j == 7),
                           skip_group_check=True)

            def dummies():
                for _ in range(NDUMMY):
                    mm(dmy[:16, :], w1[:, 0:16], w1[:, 0:64],
                       start=True, stop=True, skip_group_check=True)

            def ew_sig(fg, gb, t):
                pin("act", nc.scalar.activation(fg[t % 2][:, 0:512], gb, AF.Sigmoid))

            def ew_stt(fg, t):
                f = fg[t % 2]
                # g = 2*s - 1 (tanh-via-sigmoid fixup), in place over s
                pin("dve", nc.vector.tensor_scalar(
                    f[:, 384:512], f[:, 384:512], 2.0, 1.0, ALU.mult,
                    ALU.subtract))
                fcig = temps.tile([128, 256], dt.float16, name="fcig")
                # (g|c) * (i|f) = (ig | fc)
                pin("dve", nc.vector.tensor_mul(fcig, f[:, 384:640], f[:, 0:256]))
                pin("dve", nc.vector.tensor_add(
                    fg[(t + 1) % 2][:, 512:640], fcig[:, 0:128], fcig[:, 128:256]))

            def ew_tanh(fg, t):
                th = temps.tile([128, 128], dt.float16, name="th")
                pin("act", nc.scalar.activation(
                    th, fg[(t + 1) % 2][:, 512:640], AF.Tanh))
                return th

            _th1 = {}
            _th2 = {}

            def l1_ew_mulh(t):
                th = _th1.pop(t)
                pin("dve", nc.vector.tensor_mul(
                    h1r[t % 2][:, 0:64], fg1[t % 2][:, 256:320], th[:, 0:64]))
                pin("dve", nc.vector.tensor_mul(
                    h1r[t % 2][:, 64:128], fg1[t % 2][:, 320:384], th[:, 64:128]))

            def l2_ew_mulh(t):
                th = _th2.pop(t)
                pin("dve", nc.vector.tensor_mul(
                    h2r[t % 2][:, 0:64], fg2[t % 2][:, 256:320], th[:, 0:64]))
                pin("dve", nc.vector.tensor_mul(
                    h2r[t % 2][:, 64:128], fg2[t % 2][:, 320:384], th[:, 64:128]))
                if t == T_ - 1:
                    pin("dve", nc.vector.tensor_mul(out_sb, fg2[t % 2][:, 256:384], th))
                    nc.sync.dma_start(out=y_out[:], in_=out_sb)

            # Software pipeline, iteration tau: L1 advances to step tau+1;
            # L2's matmul head is at step tau, its elementwise lags (sigmoid
            # at tau-1, tanh/mulh at tau-2) to slot into ACT idle windows.
            l1_mm_xproj(0)
            for tau in range(-1, T_ + 2):
                if 0 <= tau + 1 < T_:
                    l1_mm_rec(tau + 1)
                if 0 <= tau < T_:
                    l2_mm_h1(tau)
                if 0 <= tau + 2 < T_:
                    l1_mm_xproj(tau + 2)
                if 0 <= tau + 1 < T_:
                    ew_sig(fg1, gb1[(tau + 1) % 4][:, :], tau + 1)
                if 0 <= tau - 2 < T_:
                    ew_stt(fg2, tau - 2)
                if 0 <= tau - 2 < T_:
                    _th2[tau - 2] = ew_tanh(fg2, tau - 2)
                if 0 <= tau + 1 < T_:
                    ew_stt(fg1, tau + 1)
                if 0 <= tau + 1 < T_:
                    _th1[tau + 1] = ew_tanh(fg1, tau + 1)
                if 0 <= tau - 2 < T_:
                    l2_ew_mulh(tau - 2)
                # h2part(tau-1) consumes h2(tau-2) - must be emitted after
                # l2_ew_mulh(tau-2) produces it (program-order dep tracking)
                if 0 <= tau - 1 < T_:
                    l2_mm_h2(tau - 1)
                if 0 <= tau < T_:
                    dummies()
                if 0 <= tau + 1 < T_:
                    l1_ew_mulh(tau + 1)
                if 0 <= tau - 1 < T_:
                    ew_sig(fg2, gb2[(tau - 1) % 2][:, :], tau - 1)

    nc.compile()
    return nc


_NC_CACHE = {}


def _get_nc(t_steps, with_b1, with_b2):
    key = (t_steps, with_b1, with_b2)
    if key not in _NC_CACHE:
        _NC_CACHE[key] = _build(t_steps, with_b1, with_b2)
    return _NC_CACHE[key]


def _pack_w(W, kchunks):
    """W [128*kchunks, 1024] -> [128, kchunks*8*128] fp16, PERM chunk order,
    with the g-gate chunk columns scaled x2 (tanh-via-sigmoid)."""
    out = np.empty((128, kchunks, 8, 128), dtype=np.float32)
    for k in range(kchunks):
        for j in range(8):
            m = PERM[j]
            w = W[128 * k : 128 * (k + 1), 128 * m : 128 * (m + 1)]
            if j in G_POS:
                w = w * 2.0
            out[:, k, j, :] = w
    return np.ascontiguousarray(out.reshape(128, kchunks * 8 * 128).astype(_F16))


def _pack_bias(b):
    """b [1024] -> [8, 128] lhsT rows in PERM order (g rows x2)."""
    bb = np.zeros((8, 128), dtype=np.float32)
    for j in range(8):
        bb[j, :] = b[128 * PERM[j] : 128 * (PERM[j] + 1)]
        if j in G_POS:
            bb[j, :] *= 2.0
    return bb.astype(_F16)


def _make_ind():
    ind = np.zeros((8, 512), dtype=_F16)
    for j in range(8):
        ind[j, 64 * j : 64 * (j + 1)] = 1
    return ind


def _pack_x_core(xc, t_steps):
    """xc [BL, T, D] f32 -> [nblk, 128, XBLK, BL] fp16 (partition = d)."""
    nblk = (t_steps + XBLK - 1) // XBLK
    xt = xc.transpose(1, 2, 0)  # [T, D, BL]
    xt = xt.reshape(nblk, XBLK, D, BL).transpose(0, 2, 1, 3)  # [nblk, D, XBLK, BL]
    return np.ascontiguousarray(xt.astype(_F16))


TRACE = False  # set by test harness to capture a HW profile
LAST_EXEC_NS = None


def kernel(x, W1, b1, W2, b2, Wout, bout):
    global LAST_EXEC_NS
    from concourse.bass_utils import run_bass_kernel_spmd

    x = np.asarray(x)
    W1 = np.asarray(W1, dtype=np.float32)
    b1 = np.asarray(b1, dtype=np.float32)
    W2 = np.asarray(W2, dtype=np.float32)
    b2 = np.asarray(b2, dtype=np.float32)
    Wout = np.asarray(Wout, dtype=np.float32)
    bout = np.asarray(bout, dtype=np.float32)
    t_steps = x.shape[1]

    with_b1 = bool(np.any(b1))
    with_b2 = bool(np.any(b2))
    nc = _get_nc(t_steps, with_b1, with_b2)

    base = {"w1": _pack_w(W1, 3), "w2": _pack_w(W2, 4)}
    if with_b1:
        base["b1"] = _pack_bias(b1)
    if with_b2:
        base["b2"] = _pack_bias(b2)
    if with_b1 or with_b2:
        base["ind"] = _make_ind()

    in_maps = []
    for i in range(NCORES):
        m = dict(base)
        m["x"] = _pack_x_core(x[i * BL : (i + 1) * BL].astype(np.float32), t_steps)
        in_maps.append(m)

    res = run_bass_kernel_spmd(nc, in_maps, list(range(NCORES)), trace=TRACE)
    LAST_EXEC_NS = res.exec_time_ns

    h2 = np.concatenate(
        [
            res.results[i]["y"].reshape(128, 2, 64).transpose(2, 1, 0).reshape(64, 256)
            for i in range(NCORES)
        ],
        axis=0,
    )
    return (h2.astype(np.float32) @ Wout + bout).astype(np.float32)


# revision 11
# speedup vs baseline: 1.0049x; 1.0049x over previous
"""Trainium2 Bass kernel for a 2-layer LSTM (B=512, T=1024, D=128, H=256, OUT=1).

Strategy: data-parallel over batch (8 cores x 64 rows). Each core runs the full
T=1024 recurrence on its batch shard. On-chip layout is "transposed": partition
dim = feature chunk (128 wide), free dim = 64*chunk_idx + batch, so h-state
tiles are directly the moving (rhs) operand of the recurrent matmuls.

v4 design (per layer, per step):
- ONE PSUM bank [128, 512] holds all 8 gate chunks in order [i i f f o o g g];
  g-gate weight columns are pre-scaled x2 on host so a SINGLE 512-wide sigmoid
  yields sigmoid for i,f,o and s = sigmoid(2*ghat) for g, using the identity
  tanh(x) = 2*sigmoid(2x) - 1.
- Cell state is stored offset+scaled: m = c/2 + 1/2 (in [0,1]), parked in the
  SAME ping-pong tile as the sigmoid output, right after the g region:
  tile layout [i | f | o | s | m] = [128, 640]. Then
    fcig = (in0 - 0.5) * in1   with in0 = [s|m], in1 = [i|f]   (one 256-wide
                               scalar_tensor_tensor: = [i*(s-.5) | f*(m-.5)])
    m_new = (fc + 0.5) + ig    (one 128-wide scalar_tensor_tensor)
    tanh(c) = Tanh(2*m - 1)    (free scale/bias on the ACT instruction)
    h = o * tanh(c)
  This is 3 DVE ops + 2 ACT ops per layer per step (v1: 4-5 DVE + 3 ACT).
- Layer 2 runs LAGGED by one pipeline iteration: its sigmoid/tanh occupy the
  ACT-engine idle slots of the L1 recurrence cycle instead of delaying it.
  Engine-queue orders are pinned with explicit deps (the Tile scheduler
  otherwise reorders ACT/DVE and puts L2 ops on the L1 critical cycle).
- A few always-ready dummy matmuls pad the PE queue right before each rec
  group: they absorb the ~190ns post-idle PE pipeline restart that otherwise
  lands on the critical path, and keep PE activity above the HAM clock-gate
  threshold (otherwise the PE oscillates between 1.2 and 2.4 GHz).
"""

import numpy as np
import ml_dtypes

B, T, D = 512, 1024, 128
H = 256
NCORES = 8
BL = B // NCORES  # 64 batch rows per core
XBLK = 16  # timesteps per x DMA block
# on-chip chunk order [i0 i1 f0 f1 o0 o1 g0 g1]; original order f i g o.
PERM = [2, 3, 0, 1, 6, 7, 4, 5]
G_POS = (6, 7)  # on-chip chunk positions holding the g gate (weights x2)
NDUMMY = 7  # PE warm-keeper matmuls per step

_F16 = np.float16


def _build(t_steps, with_b1, with_b2):
    import concourse.bass as bass  # noqa: F401
    from concourse.tile import add_dep_helper
    import concourse.mybir as mybir
    import concourse.tile as tile
    from concourse import bacc

    dt = mybir.dt
    AF = mybir.ActivationFunctionType
    ALU = mybir.AluOpType
    nblk = (t_steps + XBLK - 1) // XBLK
    T_ = t_steps

    nc = bacc.Bacc("TRN2", target_bir_lowering=False, debug=False, num_devices=NCORES)
    x_in = nc.declare_dram_parameter(
        "x", [nblk, 128, XBLK, BL], dt.float16, isOutput=False
    )
    w1_in = nc.declare_dram_parameter("w1", [128, 3 * 8 * 128], dt.float16, isOutput=False)
    w2_in = nc.declare_dram_parameter("w2", [128, 4 * 8 * 128], dt.float16, isOutput=False)
    if with_b1:
        b1_in = nc.declare_dram_parameter("b1", [8, 128], dt.float16, isOutput=False)
    if with_b2:
        b2_in = nc.declare_dram_parameter("b2", [8, 128], dt.float16, isOutput=False)
    if with_b1 or with_b2:
        ind_in = nc.declare_dram_parameter("ind", [8, 512], dt.float16, isOutput=False)
    y_out = nc.declare_dram_parameter("y", [128, 128], dt.float32, isOutput=True)

    with tile.TileContext(nc) as tc:
        with (
            tc.tile_pool(name="singles", bufs=1) as singles,
            tc.tile_pool(name="temps", bufs=8) as temps,
            tc.tile_pool(name="psum", bufs=1, space="PSUM") as psum,
        ):
            w1 = singles.tile([128, 3 * 8 * 128], dt.float16)
            w2 = singles.tile([128, 4 * 8 * 128], dt.float16)
            nc.sync.dma_start(out=w1, in_=w1_in[:])
            nc.sync.dma_start(out=w2, in_=w2_in[:])
            if with_b1:
                b1s = singles.tile([8, 128], dt.float16)
                nc.sync.dma_start(out=b1s, in_=b1_in[:])
            if with_b2:
                b2s = singles.tile([8, 128], dt.float16)
                nc.sync.dma_start(out=b2s, in_=b2_in[:])
            if with_b1 or with_b2:
                ind = singles.tile([8, 512], dt.float16)
                nc.sync.dma_start(out=ind, in_=ind_in[:])

            xr = [
                singles.tile([128, XBLK * BL], dt.float16, name=f"xr{i}")
                for i in range(3)
            ]
            h1r = [singles.tile([128, 128], dt.float16, name=f"h1r{i}") for i in range(2)]
            h2r = [singles.tile([128, 128], dt.float16, name=f"h2r{i}") for i in range(2)]
            # ping-pong [i|f|o|s|m] tiles per layer; m region [512:640]
            fg1 = [singles.tile([128, 640], dt.float16, name=f"fg1_{i}") for i in range(2)]
            fg2 = [singles.tile([128, 640], dt.float16, name=f"fg2_{i}") for i in range(2)]
            out_sb = singles.tile([128, 128], dt.float32)
            for tl in (h1r[0], h1r[1], h2r[0], h2r[1]):
                nc.gpsimd.memset(tl, 0.0)
            # c(0) = 0
            nc.gpsimd.memset(fg1[0][:, 512:640], 0.0)
            nc.gpsimd.memset(fg2[0][:, 512:640], 0.0)

            gb1 = [psum.tile([128, 512], dt.float32, name=f"gb1_{i}") for i in range(2)]
            gb2 = [psum.tile([128, 512], dt.float32, name=f"gb2_{i}") for i in range(2)]
            dmy = psum.tile([128, 64], dt.float32, name="dmy")

            nc.sync.dma_start(out=xr[0], in_=x_in[0])

            mm = nc.tensor.matmul

            def w1_tile(k, j):
                i = (k * 8 + j) * 128
                return w1[:, i : i + 128]

            def w2_tile(k, j):
                i = (k * 8 + j) * 128
                return w2[:, i : i + 128]

            def xs_of(t):
                blk = t // XBLK
                tt = t % XBLK
                return xr[blk % 3][:, tt * BL : (tt + 1) * BL]

            def dummies():
                for _ in range(NDUMMY):
                    mm(dmy[:16, :], w1[:, 0:16], w1[:, 0:64],
                       start=True, stop=True, skip_group_check=True)

            def ew(fg, gb, t, h_out, split_h):
                """fused elementwise: one sigmoid + fixup/mul/add + tanh + h"""
                f = fg[t % 2]
                nc.scalar.activation(f[:, 0:512], gb, AF.Sigmoid)
                nc.vector.tensor_scalar(
                    f[:, 384:512], f[:, 384:512], 2.0, 1.0, ALU.mult,
                    ALU.subtract)
                fcig = temps.tile([128, 256], dt.float16, name="fcig")
                nc.vector.tensor_mul(fcig, f[:, 384:640], f[:, 0:256])
                nc.vector.tensor_add(
                    fg[(t + 1) % 2][:, 512:640], fcig[:, 0:128], fcig[:, 128:256])
                th = temps.tile([128, 128], dt.float16, name="th")
                tc_inst = nc.scalar.activation(th, fg[(t + 1) % 2][:, 512:640], AF.Tanh)
                nc.vector.tensor_mul(h_out[:, 0:64], f[:, 256:320], th[:, 0:64])
                nc.vector.tensor_mul(h_out[:, 64:128], f[:, 320:384], th[:, 64:128])
                return tc_inst, th

            def emit_l1(t):
                p = t % 2
                blk = t // XBLK
                tt = t % XBLK
                if tt == 0 and blk + 1 < nblk:
                    nc.sync.dma_start(out=xr[(blk + 1) % 3], in_=x_in[blk + 1])
                xs = xs_of(t)
                dummies()
                for j in range(8):
                    mm(gb1[p][:, 64 * j : 64 * j + 64], w1_tile(0, j), xs,
                       start=(j == 0), stop=False, skip_group_check=True)
                if with_b1:
                    mm(gb1[p][:, :], b1s, ind, start=False, stop=False,
                       skip_group_check=True)
                h1_prev = h1r[(t + 1) % 2]
                for k in (1, 2):
                    hk = h1_prev[:, 64 * (k - 1) : 64 * k]
                    for j in range(8):
                        mm(gb1[p][:, 64 * j : 64 * j + 64], w1_tile(k, j), hk,
                           start=False, stop=(k == 2 and j == 7),
                           skip_group_check=True)
                tc_inst, _ = ew(fg1, gb1[p][:, :], t, h1r[t % 2], True)
                return tc_inst

            def emit_l2(t, tc1_inst=None):
                p = t % 2
                h1_cur = h1r[t % 2]
                h2_prev = h2r[(t + 1) % 2]
                for k in (0, 1):
                    hk = h1_cur[:, 64 * k : 64 * (k + 1)]
                    for j in range(8):
                        mm(gb2[p][:, 64 * j : 64 * j + 64], w2_tile(k, j), hk,
                           start=(k == 0 and j == 0), stop=False,
                           skip_group_check=True)
                if with_b2:
                    mm(gb2[p][:, :], b2s, ind, start=False, stop=False,
                       skip_group_check=True)
                for k in (2, 3):
                    hk = h2_prev[:, 64 * (k - 2) : 64 * (k - 1)]
                    for j in range(8):
                        mm(gb2[p][:, 64 * j : 64 * j + 64], w2_tile(k, j), hk,
                           start=False, stop=(k == 3 and j == 7),
                           skip_group_check=True)
                f = fg2[t % 2]
                s2_inst = nc.scalar.activation(f[:, 0:512], gb2[p][:, :], AF.Sigmoid)
                if tc1_inst is not None:
                    # keep next step's tanh(c1) ahead of this step's big L2
                    # sigmoid in the ACT FIFO (it is on the h1 recurrence cycle)
                    add_dep_helper(s2_inst.ins, tc1_inst.ins,
                                   reason="h1-cycle tanh_c before L2 sigmoid")
                nc.vector.tensor_scalar(
                    f[:, 384:512], f[:, 384:512], 2.0, 1.0, ALU.mult,
                    ALU.subtract)
                fcig = temps.tile([128, 256], dt.float16, name="fcig2")
                nc.vector.tensor_mul(fcig, f[:, 384:640], f[:, 0:256])
                nc.vector.tensor_add(
                    fg2[(t + 1) % 2][:, 512:640], fcig[:, 0:128], fcig[:, 128:256])
                th = temps.tile([128, 128], dt.float16, name="th2")
                nc.scalar.activation(th, fg2[(t + 1) % 2][:, 512:640], AF.Tanh)
                nc.vector.tensor_mul(h2r[t % 2][:, 0:64], f[:, 256:320], th[:, 0:64])
                nc.vector.tensor_mul(h2r[t % 2][:, 64:128], f[:, 320:384], th[:, 64:128])
                if t == T_ - 1:
                    nc.vector.tensor_mul(out_sb, f[:, 256:384], th)
                    nc.sync.dma_start(out=y_out[:], in_=out_sb)

            # v1-proven software pipeline: L1 of step tau+1 is emitted before
            # L2 of step tau so the PE work between h1(tau) and L1rec(tau+1)
            # is minimal.
            emit_l1(0)
            for tau in range(T_):
                tc1 = emit_l1(tau + 1) if tau + 1 < T_ else None
                emit_l2(tau, tc1)

    nc.compile()
    return nc


_NC_CACHE = {}


def _get_nc(t_steps, with_b1, with_b2):
    key = (t_steps, with_b1, with_b2)
    if key not in _NC_CACHE:
        _NC_CACHE[key] = _build(t_steps, with_b1, with_b2)
    return _NC_CACHE[key]


def _pack_w(W, kchunks):
    """W [128*kchunks, 1024] -> [128, kchunks*8*128] fp16, PERM chunk order,
    with the g-gate chunk columns scaled x2 (tanh-via-sigmoid)."""
    out = np.empty((128, kchunks, 8, 128), dtype=np.float32)
    for k in range(kchunks):
        for j in range(8):
            m = PERM[j]
            w = W[128 * k : 128 * (k + 1), 128 * m : 128 * (m + 1)]
            if j in G_POS:
                w = w * 2.0
            out[:, k, j, :] = w
    return np.ascontiguousarray(out.reshape(128, kchunks * 8 * 128).astype(_F16))


def _pack_bias(b):
    """b [1024] -> [8, 128] lhsT rows in PERM order (g rows x2)."""
    bb = np.zeros((8, 128), dtype=np.float32)
    for j in range(8):
        bb[j, :] = b[128 * PERM[j] : 128 * (PERM[j] + 1)]
        if j in G_POS:
            bb[j, :] *= 2.0
    return bb.astype(_F16)


def _make_ind():
    ind = np.zeros((8, 512), dtype=_F16)
    for j in range(8):
        ind[j, 64 * j : 64 * (j + 1)] = 1
    return ind


def _pack_x_core(xc, t_steps):
    """xc [BL, T, D] f32 -> [nblk, 128, XBLK, BL] fp16 (partition = d)."""
    nblk = (t_steps + XBLK - 1) // XBLK
    xt = xc.transpose(1, 2, 0)  # [T, D, BL]
    xt = xt.reshape(nblk, XBLK, D, BL).transpose(0, 2, 1, 3)  # [nblk, D, XBLK, BL]
    return np.ascontiguousarray(xt.astype(_F16))


TRACE = False  # set by test harness to capture a HW profile
LAST_EXEC_NS = None


def kernel(x, W1, b1, W2, b2, Wout, bout):
    global LAST_EXEC_NS
    from concourse.bass_utils import run_bass_kernel_spmd

    x = np.asarray(x)
    W1 = np.asarray(W1, dtype=np.float32)
    b1 = np.asarray(b1, dtype=np.float32)
    W2 = np.asarray(W2, dtype=np.float32)
    b2 = np.asarray(b2, dtype=np.float32)
    Wout = np.asarray(Wout, dtype=np.float32)
    bout = np.asarray(bout, dtype=np.float32)
    t_steps = x.shape[1]

    with_b1 = bool(np.any(b1))
    with_b2 = bool(np.any(b2))
    nc = _get_nc(t_steps, with_b1, with_b2)

    base = {"w1": _pack_w(W1, 3), "w2": _pack_w(W2, 4)}
    if with_b1:
        base["b1"] = _pack_bias(b1)
    if with_b2:
        base["b2"] = _pack_bias(b2)
    if with_b1 or with_b2:
        base["ind"] = _make_ind()

    in_maps = []
    for i in range(NCORES):
        m = dict(base)
        m["x"] = _pack_x_core(x[i * BL : (i + 1) * BL].astype(np.float32), t_steps)
        in_maps.append(m)

    res = run_bass_kernel_spmd(nc, in_maps, list(range(NCORES)), trace=TRACE)
    LAST_EXEC_NS = res.exec_time_ns

    h2 = np.concatenate(
        [
            res.results[i]["y"].reshape(128, 2, 64).transpose(2, 1, 0).reshape(64, 256)
            for i in range(NCORES)
        ],
        axis=0,
    )
    return (h2.astype(np.float32) @ Wout + bout).astype(np.float32)


# revision 13
# speedup vs baseline: 1.0055x; 1.0006x over previous
"""Trainium2 Bass kernel for a 2-layer LSTM (B=512, T=1024, D=128, H=256, OUT=1).

Strategy: data-parallel over batch (8 cores x 64 rows). Each core runs the full
T=1024 recurrence on its batch shard. On-chip layout is "transposed": partition
dim = feature chunk (128 wide), free dim = 64*chunk_idx + batch, so h-state
tiles are directly the moving (rhs) operand of the recurrent matmuls.

v4 design (per layer, per step):
- ONE PSUM bank [128, 512] holds all 8 gate chunks in order [i i f f o o g g];
  g-gate weight columns are pre-scaled x2 on host so a SINGLE 512-wide sigmoid
  yields sigmoid for i,f,o and s = sigmoid(2*ghat) for g, using the identity
  tanh(x) = 2*sigmoid(2x) - 1.
- Cell state is stored offset+scaled: m = c/2 + 1/2 (in [0,1]), parked in the
  SAME ping-pong tile as the sigmoid output, right after the g region:
  tile layout [i | f | o | s | m] = [128, 640]. Then
    fcig = (in0 - 0.5) * in1   with in0 = [s|m], in1 = [i|f]   (one 256-wide
                               scalar_tensor_tensor: = [i*(s-.5) | f*(m-.5)])
    m_new = (fc + 0.5) + ig    (one 128-wide scalar_tensor_tensor)
    tanh(c) = Tanh(2*m - 1)    (free scale/bias on the ACT instruction)
    h = o * tanh(c)
  This is 3 DVE ops + 2 ACT ops per layer per step (v1: 4-5 DVE + 3 ACT).
- Layer 2 runs LAGGED by one pipeline iteration: its sigmoid/tanh occupy the
  ACT-engine idle slots of the L1 recurrence cycle instead of delaying it.
  Engine-queue orders are pinned with explicit deps (the Tile scheduler
  otherwise reorders ACT/DVE and puts L2 ops on the L1 critical cycle).
- A few always-ready dummy matmuls pad the PE queue right before each rec
  group: they absorb the ~190ns post-idle PE pipeline restart that otherwise
  lands on the critical path, and keep PE activity above the HAM clock-gate
  threshold (otherwise the PE oscillates between 1.2 and 2.4 GHz).
"""

import numpy as np
import ml_dtypes

B, T, D = 512, 1024, 128
H = 256
NCORES = 8
BL = B // NCORES  # 64 batch rows per core
XBLK = 16  # timesteps per x DMA block
# on-chip chunk order [f0 f1 i0 i1 o0 o1 g0 g1]; original order f i g o.
PERM = [0, 1, 2, 3, 6, 7, 4, 5]
G_POS = (6, 7)  # on-chip chunk positions holding the g gate (weights x2)
NDUMMY = 7  # PE warm-keeper matmuls per step

_F16 = np.float16


def _build(t_steps, with_b1, with_b2):
    import concourse.bass as bass  # noqa: F401
    from concourse.tile import add_dep_helper
    import concourse.mybir as mybir
    import concourse.tile as tile
    from concourse import bacc

    dt = mybir.dt
    AF = mybir.ActivationFunctionType
    ALU = mybir.AluOpType
    nblk = (t_steps + XBLK - 1) // XBLK
    T_ = t_steps

    nc = bacc.Bacc("TRN2", target_bir_lowering=False, debug=False, num_devices=NCORES)
    x_in = nc.declare_dram_parameter(
        "x", [nblk, 128, XBLK, BL], dt.float16, isOutput=False
    )
    w1_in = nc.declare_dram_parameter("w1", [128, 3 * 8 * 128], dt.bfloat16, isOutput=False)
    w2_in = nc.declare_dram_parameter("w2", [128, 4 * 8 * 128], dt.bfloat16, isOutput=False)
    if with_b1:
        b1_in = nc.declare_dram_parameter("b1", [8, 128], dt.bfloat16, isOutput=False)
    if with_b2:
        b2_in = nc.declare_dram_parameter("b2", [8, 128], dt.bfloat16, isOutput=False)
    if with_b1 or with_b2:
        ind_in = nc.declare_dram_parameter("ind", [8, 512], dt.float16, isOutput=False)
    y_out = nc.declare_dram_parameter("y", [128, 128], dt.float32, isOutput=True)

    with tile.TileContext(nc) as tc:
        with (
            tc.tile_pool(name="singles", bufs=1) as singles,
            tc.tile_pool(name="temps", bufs=8) as temps,
            tc.tile_pool(name="psum", bufs=1, space="PSUM") as psum,
        ):
            w1 = singles.tile([128, 3 * 8 * 128], dt.bfloat16)
            w2 = singles.tile([128, 4 * 8 * 128], dt.bfloat16)
            nc.sync.dma_start(out=w1, in_=w1_in[:])
            nc.sync.dma_start(out=w2, in_=w2_in[:])
            if with_b1:
                b1s = singles.tile([8, 128], dt.bfloat16)
                nc.sync.dma_start(out=b1s, in_=b1_in[:])
            if with_b2:
                b2s = singles.tile([8, 128], dt.bfloat16)
                nc.sync.dma_start(out=b2s, in_=b2_in[:])
            if with_b1 or with_b2:
                ind = singles.tile([8, 512], dt.float16)
                nc.sync.dma_start(out=ind, in_=ind_in[:])

            xr = [
                singles.tile([128, XBLK * BL], dt.float16, name=f"xr{i}")
                for i in range(3)
            ]
            h1r = [singles.tile([128, 128], dt.float16, name=f"h1r{i}") for i in range(2)]
            h2r = [singles.tile([128, 128], dt.float16, name=f"h2r{i}") for i in range(2)]
            cg1 = singles.tile([128, 256], dt.float16)  # [c | tanh(g)] co-tile
            cg2 = singles.tile([128, 256], dt.float16)
            out_sb = singles.tile([128, 128], dt.float32)
            for tl in (h1r[0], h1r[1], h2r[0], h2r[1], cg1, cg2):
                nc.gpsimd.memset(tl, 0.0)

            gb1 = [psum.tile([128, 512], dt.float32, name=f"gb1_{i}") for i in range(2)]
            gb2 = [psum.tile([128, 512], dt.float32, name=f"gb2_{i}") for i in range(2)]
            dmy = psum.tile([128, 64], dt.float32, name="dmy")

            nc.sync.dma_start(out=xr[0], in_=x_in[0])

            mm = nc.tensor.matmul

            def w1_tile(k, j):
                i = (k * 8 + j) * 128
                return w1[:, i : i + 128]

            def w2_tile(k, j):
                i = (k * 8 + j) * 128
                return w2[:, i : i + 128]

            def xs_of(t):
                blk = t // XBLK
                tt = t % XBLK
                return xr[blk % 3][:, tt * BL : (tt + 1) * BL]

            def dummies():
                for _ in range(NDUMMY):
                    mm(dmy[:16, :], w1[:, 0:16], w1[:, 0:64],
                       start=True, stop=True, skip_group_check=True)

            def ew(cg, gb, t, h_out, name):
                """fused elementwise: one sigmoid + fixup/mul/add + tanh + h"""
                figog = temps.tile([128, 512], dt.float16, name="figog" + name)
                sig_inst = nc.scalar.activation(figog, gb, AF.Sigmoid)
                # g = 2*sigmoid(2*ghat) - 1 == tanh(ghat)
                nc.vector.tensor_scalar(
                    cg[:, 128:256], figog[:, 384:512], 2.0, 1.0, ALU.mult,
                    ALU.subtract)
                fcig = temps.tile([128, 256], dt.float16, name="fcig" + name)
                # (f|i) * (c|g) = (fc | ig)
                nc.vector.tensor_mul(fcig, figog[:, 0:256], cg)
                nc.vector.tensor_add(cg[:, 0:128], fcig[:, 0:128], fcig[:, 128:256])
                th = temps.tile([128, 128], dt.float16, name="th" + name)
                tc_inst = nc.scalar.activation(th, cg[:, 0:128], AF.Tanh)
                nc.vector.tensor_mul(h_out[:, 0:64], figog[:, 256:320], th[:, 0:64])
                nc.vector.tensor_mul(h_out[:, 64:128], figog[:, 320:384], th[:, 64:128])
                return tc_inst, sig_inst, figog, th

            def emit_l1(t):
                p = t % 2
                blk = t // XBLK
                tt = t % XBLK
                if tt == 0 and blk + 1 < nblk:
                    nc.sync.dma_start(out=xr[(blk + 1) % 3], in_=x_in[blk + 1])
                xs = xs_of(t)
                dummies()
                for j in range(8):
                    mm(gb1[p][:, 64 * j : 64 * j + 64], w1_tile(0, j), xs,
                       start=(j == 0), stop=False, skip_group_check=True)
                if with_b1:
                    mm(gb1[p][:, :], b1s, ind, start=False, stop=False,
                       skip_group_check=True)
                h1_prev = h1r[(t + 1) % 2]
                for k in (1, 2):
                    hk = h1_prev[:, 64 * (k - 1) : 64 * k]
                    for j in range(8):
                        mm(gb1[p][:, 64 * j : 64 * j + 64], w1_tile(k, j), hk,
                           start=False, stop=(k == 2 and j == 7),
                           skip_group_check=True)
                tc_inst, _, _, _ = ew(cg1, gb1[p][:, :], t, h1r[t % 2], "1")
                return tc_inst

            def emit_l2(t, tc1_inst=None):
                p = t % 2
                h1_cur = h1r[t % 2]
                h2_prev = h2r[(t + 1) % 2]
                for k in (0, 1):
                    hk = h1_cur[:, 64 * k : 64 * (k + 1)]
                    for j in range(8):
                        mm(gb2[p][:, 64 * j : 64 * j + 64], w2_tile(k, j), hk,
                           start=(k == 0 and j == 0), stop=False,
                           skip_group_check=True)
                if with_b2:
                    mm(gb2[p][:, :], b2s, ind, start=False, stop=False,
                       skip_group_check=True)
                for k in (2, 3):
                    hk = h2_prev[:, 64 * (k - 2) : 64 * (k - 1)]
                    for j in range(8):
                        mm(gb2[p][:, 64 * j : 64 * j + 64], w2_tile(k, j), hk,
                           start=False, stop=(k == 3 and j == 7),
                           skip_group_check=True)
                _, s2_inst, figog, th = ew(cg2, gb2[p][:, :], t, h2r[t % 2], "2")
                if tc1_inst is not None:
                    # keep next step's tanh(c1) ahead of this step's big L2
                    # sigmoid in the ACT FIFO (it is on the h1 recurrence cycle)
                    add_dep_helper(s2_inst.ins, tc1_inst.ins,
                                   reason="h1-cycle tanh_c before L2 sigmoid")
                if t == T_ - 1:
                    nc.vector.tensor_mul(out_sb, figog[:, 256:384], th)
                    nc.sync.dma_start(out=y_out[:], in_=out_sb)

            # v1-proven software pipeline: L1 of step tau+1 is emitted before
            # L2 of step tau so the PE work between h1(tau) and L1rec(tau+1)
            # is minimal.
            emit_l1(0)
            for tau in range(T_):
                tc1 = emit_l1(tau + 1) if tau + 1 < T_ else None
                emit_l2(tau, tc1)

    nc.compile()
    return nc


_NC_CACHE = {}


def _get_nc(t_steps, with_b1, with_b2):
    key = (t_steps, with_b1, with_b2)
    if key not in _NC_CACHE:
        _NC_CACHE[key] = _build(t_steps, with_b1, with_b2)
    return _NC_CACHE[key]


def _pack_w(W, kchunks):
    """W [128*kchunks, 1024] -> [128, kchunks*8*128] fp16, PERM chunk order,
    with the g-gate chunk columns scaled x2 (tanh-via-sigmoid)."""
    out = np.empty((128, kchunks, 8, 128), dtype=np.float32)
    for k in range(kchunks):
        for j in range(8):
            m = PERM[j]
            w = W[128 * k : 128 * (k + 1), 128 * m : 128 * (m + 1)]
            if j in G_POS:
                w = w * 2.0
            out[:, k, j, :] = w
    return np.ascontiguousarray(out.reshape(128, kchunks * 8 * 128).astype(ml_dtypes.bfloat16))


def _pack_bias(b):
    """b [1024] -> [8, 128] lhsT rows in PERM order (g rows x2)."""
    bb = np.zeros((8, 128), dtype=np.float32)
    for j in range(8):
        bb[j, :] = b[128 * PERM[j] : 128 * (PERM[j] + 1)]
        if j in G_POS:
            bb[j, :] *= 2.0
    return bb.astype(ml_dtypes.bfloat16)


def _make_ind():
    ind = np.zeros((8, 512), dtype=_F16)
    for j in range(8):
        ind[j, 64 * j : 64 * (j + 1)] = 1
    return ind


def _pack_x_core(xc, t_steps):
    """xc [BL, T, D] f32 -> [nblk, 128, XBLK, BL] fp16 (partition = d)."""
    nblk = (t_steps + XBLK - 1) // XBLK
    xt = xc.transpose(1, 2, 0)  # [T, D, BL]
    xt = xt.reshape(nblk, XBLK, D, BL).transpose(0, 2, 1, 3)  # [nblk, D, XBLK, BL]
    return np.ascontiguousarray(xt.astype(_F16))


TRACE = False  # set by test harness to capture a HW profile
LAST_EXEC_NS = None


def kernel(x, W1, b1, W2, b2, Wout, bout):
    global LAST_EXEC_NS
    from concourse.bass_utils import run_bass_kernel_spmd

    x = np.asarray(x)
    W1 = np.asarray(W1, dtype=np.float32)
    b1 = np.asarray(b1, dtype=np.float32)
    W2 = np.asarray(W2, dtype=np.float32)
    b2 = np.asarray(b2, dtype=np.float32)
    Wout = np.asarray(Wout, dtype=np.float32)
    bout = np.asarray(bout, dtype=np.float32)
    t_steps = x.shape[1]

    with_b1 = bool(np.any(b1))
    with_b2 = bool(np.any(b2))
    nc = _get_nc(t_steps, with_b1, with_b2)

    base = {"w1": _pack_w(W1, 3), "w2": _pack_w(W2, 4)}
    if with_b1:
        base["b1"] = _pack_bias(b1)
    if with_b2:
        base["b2"] = _pack_bias(b2)
    if with_b1 or with_b2:
        base["ind"] = _make_ind()

    in_maps = []
    for i in range(NCORES):
        m = dict(base)
        m["x"] = _pack_x_core(x[i * BL : (i + 1) * BL].astype(np.float32), t_steps)
        in_maps.append(m)

    res = run_bass_kernel_spmd(nc, in_maps, list(range(NCORES)), trace=TRACE)
    LAST_EXEC_NS = res.exec_time_ns

    h2 = np.concatenate(
        [
            res.results[i]["y"].reshape(128, 2, 64).transpose(2, 1, 0).reshape(64, 256)
            for i in range(NCORES)
        ],
        axis=0,
    )
    return (h2.astype(np.float32) @ Wout + bout).astype(np.float32)


# revision 14
# speedup vs baseline: 1.3764x; 1.3688x over previous
"""Trainium2 Bass kernel for a 2-layer LSTM (B=512, T=1024, D=128, H=256, OUT=1).

Strategy: data-parallel over batch (8 cores x 64 rows). Each core runs the full
T=1024 recurrence on its batch shard. All tensors on-chip use a "transposed"
layout: partition dim = feature dim chunk (128 wide), free dim = 64*chunk_idx +
batch. In this layout the h-state tiles are directly usable as the moving (rhs)
operand of the recurrent matmuls (weights stationary), so no per-step
transposes are needed anywhere.

Per step and per layer, the 4H=1024 gate dims form 8 chunks of 128. Chunks are
permuted so the sigmoid gates (f, i, o) land in one PSUM bank ([128, 384]) and
the tanh gate (g) in another ([128, 128]); each bank accumulates
x-projection + recurrent matmuls via the per-element has_written PSUM
mechanism (single start=True per bank per step). Gate activations then read
each bank with one wide ACT instruction. Banks ping-pong across steps
(2 layers x 2 banks x 2 = 8 banks = all of PSUM).

The final projection (h2_T @ Wout + bout, OUT=1) is numerically trivial and is
done on host after gathering the per-core final h2.
"""

import numpy as np
import ml_dtypes

B, T, D = 512, 1024, 128
H = 256
NCORES = 8
BL = B // NCORES  # 64 batch rows per core
XBLK = 16  # timesteps per x DMA block
# gate chunk permutation: original 4H chunk order is f(0,1) i(2,3) g(4,5) o(6,7);
# on-chip order is [f0 f1 i0 i1 o0 o1 | g0 g1] so sigmoid gates are contiguous.
PERM = [0, 1, 2, 3, 6, 7, 4, 5]

_BF16 = ml_dtypes.bfloat16
EW_BF16 = True  # bf16 elementwise datapath (2x DVE modes); False = fp32


def _build(t_steps, with_b1, with_b2, ew_bf16=None):
    import concourse.bass as bass  # noqa: F401
    from concourse.tile import add_dep_helper
    import concourse.mybir as mybir
    import concourse.tile as tile
    from concourse import bacc

    dt = mybir.dt
    AF = mybir.ActivationFunctionType
    nblk = (t_steps + XBLK - 1) // XBLK

    if ew_bf16 is None:
        ew_bf16 = EW_BF16
    global EW_BF16_ACTIVE
    nc = bacc.Bacc("TRN2", target_bir_lowering=False, debug=False, num_devices=NCORES)
    x_in = nc.declare_dram_parameter(
        "x", [nblk, 128, XBLK, BL], dt.bfloat16, isOutput=False
    )
    w1_in = nc.declare_dram_parameter("w1", [128, 3 * 8 * 128], dt.bfloat16, isOutput=False)
    w2_in = nc.declare_dram_parameter("w2", [128, 4 * 8 * 128], dt.bfloat16, isOutput=False)
    if with_b1:
        b1f_in = nc.declare_dram_parameter("b1f", [6, 128], dt.bfloat16, isOutput=False)
        b1g_in = nc.declare_dram_parameter("b1g", [2, 128], dt.bfloat16, isOutput=False)
    if with_b2:
        b2f_in = nc.declare_dram_parameter("b2f", [6, 128], dt.bfloat16, isOutput=False)
        b2g_in = nc.declare_dram_parameter("b2g", [2, 128], dt.bfloat16, isOutput=False)
    if with_b1 or with_b2:
        indf_in = nc.declare_dram_parameter("indf", [6, 384], dt.bfloat16, isOutput=False)
        indg_in = nc.declare_dram_parameter("indg", [2, 128], dt.bfloat16, isOutput=False)
    y_out = nc.declare_dram_parameter("y", [128, 128], dt.float32, isOutput=True)

    with tile.TileContext(nc) as tc:
        with (
            tc.tile_pool(name="singles", bufs=1) as singles,
            tc.tile_pool(name="temps", bufs=6) as temps,
            tc.tile_pool(name="psum", bufs=1, space="PSUM") as psum,
        ):
            w1 = singles.tile([128, 3 * 8 * 128], dt.bfloat16)
            w2 = singles.tile([128, 4 * 8 * 128], dt.bfloat16)
            nc.sync.dma_start(out=w1, in_=w1_in[:])
            nc.sync.dma_start(out=w2, in_=w2_in[:])
            if with_b1:
                b1f = singles.tile([6, 128], dt.bfloat16)
                b1g = singles.tile([2, 128], dt.bfloat16)
                nc.sync.dma_start(out=b1f, in_=b1f_in[:])
                nc.sync.dma_start(out=b1g, in_=b1g_in[:])
            if with_b2:
                b2f = singles.tile([6, 128], dt.bfloat16)
                b2g = singles.tile([2, 128], dt.bfloat16)
                nc.sync.dma_start(out=b2f, in_=b2f_in[:])
                nc.sync.dma_start(out=b2g, in_=b2g_in[:])
            if with_b1 or with_b2:
                indf = singles.tile([6, 384], dt.bfloat16)
                indg = singles.tile([2, 128], dt.bfloat16)
                nc.sync.dma_start(out=indf, in_=indf_in[:])
                nc.sync.dma_start(out=indg, in_=indg_in[:])

            xr = [
                singles.tile([128, XBLK * BL], dt.bfloat16, name=f"xr{i}")
                for i in range(3)
            ]
            h1r = [singles.tile([128, 128], dt.bfloat16, name=f"h1r{i}") for i in range(2)]
            h2r = [singles.tile([128, 128], dt.bfloat16, name=f"h2r{i}") for i in range(2)]
            ew_dt = dt.bfloat16 if ew_bf16 else dt.float32
            cg1 = singles.tile([128, 256], ew_dt)  # [c | tanh(g)] co-tile
            cg2 = singles.tile([128, 256], ew_dt)
            out_sb = singles.tile([128, 128], dt.float32)
            for tl in (h1r[0], h1r[1], h2r[0], h2r[1], cg1, cg2):
                nc.gpsimd.memset(tl, 0.0)

            g1f = [psum.tile([128, 384], dt.float32, name=f"g1f{i}") for i in range(2)]
            g1g = [psum.tile([128, 128], dt.float32, name=f"g1g{i}") for i in range(2)]
            g2f = [psum.tile([128, 384], dt.float32, name=f"g2f{i}") for i in range(2)]
            g2g = [psum.tile([128, 128], dt.float32, name=f"g2g{i}") for i in range(2)]

            nc.sync.dma_start(out=xr[0], in_=x_in[0])

            mm = nc.tensor.matmul

            def w1_tile(k, j):
                i = (k * 8 + j) * 128
                return w1[:, i : i + 128]

            def w2_tile(k, j):
                i = (k * 8 + j) * 128
                return w2[:, i : i + 128]

            def xs_of(t):
                blk = t // XBLK
                tt = t % XBLK
                return xr[blk % 3][:, tt * BL : (tt + 1) * BL]

            def emit_l1(t):
                """x-projection + L1 recurrent matmuls + L1 elementwise -> h1(t).

                Critical-cycle code: keep the PE prefix (just xproj+L1rec) as
                short as possible; L2 matmuls of step t-1 are emitted after
                this so they fill the chain's PE-idle window.
                """
                p = t % 2
                blk = t // XBLK
                tt = t % XBLK
                if tt == 0 and blk + 1 < nblk:
                    nc.sync.dma_start(out=xr[(blk + 1) % 3], in_=x_in[blk + 1])
                xs = xs_of(t)
                h1_prev = h1r[(t + 1) % 2]
                for j in range(2):  # x-projection, g bank
                    mm(g1g[p][:, 64 * j : 64 * j + 64], w1_tile(0, 6 + j), xs,
                       start=(j == 0), stop=False, skip_group_check=True)
                for j in range(6):  # x-projection, figo bank
                    mm(g1f[p][:, 64 * j : 64 * j + 64], w1_tile(0, j), xs,
                       start=(j == 0), stop=False, skip_group_check=True)
                if with_b1:
                    mm(g1g[p][:, 0:128], b1g, indg, start=False, stop=False,
                       skip_group_check=True)
                    mm(g1f[p][:, 0:384], b1f, indf, start=False, stop=False,
                       skip_group_check=True)
                for k in (1, 2):  # recurrent, g bank first (tanh can start early)
                    hk = h1_prev[:, 64 * (k - 1) : 64 * k]
                    for j in range(2):
                        mm(g1g[p][:, 64 * j : 64 * j + 64], w1_tile(k, 6 + j), hk,
                           start=False, stop=(k == 2 and j == 1), skip_group_check=True)
                for k in (1, 2):
                    hk = h1_prev[:, 64 * (k - 1) : 64 * k]
                    for j in range(6):
                        mm(g1f[p][:, 64 * j : 64 * j + 64], w1_tile(k, j), hk,
                           start=False, stop=(k == 2 and j == 5), skip_group_check=True)
                # elementwise: figo sigmoid first (it is on the h1 cycle),
                # then cg1 right half <- tanh(g); then fused f*c | i*g
                figo1 = temps.tile([128, 384], ew_dt, name="figo1")
                nc.scalar.activation(figo1, g1f[p][:, :], AF.Sigmoid)
                nc.scalar.activation(cg1[:, 128:256], g1g[p][:, :], AF.Tanh)
                fcig1 = temps.tile([128, 256], ew_dt, name="fcig1")
                nc.vector.tensor_mul(fcig1, figo1[:, 0:256], cg1)
                nc.vector.tensor_add(cg1[:, 0:128], fcig1[:, 0:128], fcig1[:, 128:256])
                th1 = temps.tile([128, 128], ew_dt, name="th1")
                tc1_inst = nc.scalar.activation(th1, cg1[:, 0:128], AF.Tanh)
                nc.vector.tensor_mul(h1r[t % 2][:, 0:64], figo1[:, 256:320], th1[:, 0:64])
                nc.vector.tensor_mul(h1r[t % 2][:, 64:128], figo1[:, 320:384], th1[:, 64:128])
                return tc1_inst

            def emit_l2(t, tc1_inst=None):
                """L2 matmuls (h1 part leads the bank group) + elementwise -> h2(t)."""
                p = t % 2
                h1_cur = h1r[t % 2]
                h2_prev = h2r[(t + 1) % 2]
                for k in (0, 1):  # h1-dependent part first: group leader (start=True)
                    hk = h1_cur[:, 64 * k : 64 * (k + 1)]
                    for j in range(2):
                        mm(g2g[p][:, 64 * j : 64 * j + 64], w2_tile(k, 6 + j), hk,
                           start=(k == 0 and j == 0), stop=False, skip_group_check=True)
                for k in (0, 1):
                    hk = h1_cur[:, 64 * k : 64 * (k + 1)]
                    for j in range(6):
                        mm(g2f[p][:, 64 * j : 64 * j + 64], w2_tile(k, j), hk,
                           start=(k == 0 and j == 0), stop=False, skip_group_check=True)
                if with_b2:
                    mm(g2g[p][:, 0:128], b2g, indg, start=False, stop=False,
                       skip_group_check=True)
                    mm(g2f[p][:, 0:384], b2f, indf, start=False, stop=False,
                       skip_group_check=True)
                for k in (2, 3):  # h2-dependent part (ready since last step)
                    hk = h2_prev[:, 64 * (k - 2) : 64 * (k - 1)]
                    for j in range(2):
                        mm(g2g[p][:, 64 * j : 64 * j + 64], w2_tile(k, 6 + j), hk,
                           start=False, stop=(k == 3 and j == 1), skip_group_check=True)
                for k in (2, 3):
                    hk = h2_prev[:, 64 * (k - 2) : 64 * (k - 1)]
                    for j in range(6):
                        mm(g2f[p][:, 64 * j : 64 * j + 64], w2_tile(k, j), hk,
                           start=False, stop=(k == 3 and j == 5), skip_group_check=True)
                nc.scalar.activation(cg2[:, 128:256], g2g[p][:, :], AF.Tanh)
                figo2 = temps.tile([128, 384], ew_dt, name="figo2")
                f2_inst = nc.scalar.activation(figo2, g2f[p][:, :], AF.Sigmoid)
                if tc1_inst is not None:
                    # keep next step's tanh(c1) ahead of this step's big L2
                    # sigmoid in the ACT FIFO: tanh(c1) is on the h1 recurrence
                    # cycle, figo2 is not.
                    add_dep_helper(f2_inst.ins, tc1_inst.ins,
                                   reason="h1-cycle tanh_c before L2 sigmoid")
                fcig2 = temps.tile([128, 256], ew_dt, name="fcig2")
                nc.vector.tensor_mul(fcig2, figo2[:, 0:256], cg2)
                nc.vector.tensor_add(cg2[:, 0:128], fcig2[:, 0:128], fcig2[:, 128:256])
                th2 = temps.tile([128, 128], ew_dt, name="th2")
                nc.scalar.activation(th2, cg2[:, 0:128], AF.Tanh)
                nc.vector.tensor_mul(h2r[t % 2], figo2[:, 256:384], th2)
                if t == t_steps - 1:
                    nc.vector.tensor_mul(out_sb, figo2[:, 256:384], th2)
                    nc.sync.dma_start(out=y_out[:], in_=out_sb)

            # software pipeline: L1 of step tau+1 is emitted before L2 of step
            # tau, so the PE work between h1(tau) and L1rec(tau+1) is minimal.
            emit_l1(0)
            for tau in range(t_steps):
                tc1 = emit_l1(tau + 1) if tau + 1 < t_steps else None
                emit_l2(tau, tc1)

    nc.compile()
    return nc


_NC_CACHE = {}


def _get_nc(t_steps, with_b1, with_b2):
    key = (t_steps, with_b1, with_b2, EW_BF16)
    if key not in _NC_CACHE:
        _NC_CACHE[key] = _build(t_steps, with_b1, with_b2, EW_BF16)
    return _NC_CACHE[key]


def _pack_w(W, kchunks):
    """W [128*kchunks, 1024] -> [128, kchunks*8*128] bf16 with PERM chunk order."""
    out = np.empty((128, kchunks, 8, 128), dtype=_BF16)
    for k in range(kchunks):
        for j in range(8):
            m = PERM[j]
            out[:, k, j, :] = W[128 * k : 128 * (k + 1), 128 * m : 128 * (m + 1)].astype(
                _BF16
            )
    return np.ascontiguousarray(out.reshape(128, kchunks * 8 * 128))


def _pack_bias(b):
    """b [1024] -> lhsT tiles for the bias matmuls.

    Bias matmul: out[p, n] += sum_k lhsT[k, p] * ind[k, n], out partition p in
    0..127, n = 64*j + bcol. ind[k, n] = delta(k, j(n)). Want out[p, 64j+bcol]
    = b[128*PERM[j] + p] -> lhsT[j, p] = b[128*PERM[j] + p].
    lhsT free size must equal out partition size (128).
    """
    bf = np.zeros((6, 128), dtype=_BF16)
    bg = np.zeros((2, 128), dtype=_BF16)
    for j in range(6):
        bf[j, :] = b[128 * PERM[j] : 128 * (PERM[j] + 1)].astype(_BF16)
    for j in range(2):
        bg[j, :] = b[128 * PERM[6 + j] : 128 * (PERM[6 + j] + 1)].astype(_BF16)
    return bf, bg


def _make_indicators():
    indf = np.zeros((6, 384), dtype=_BF16)
    indg = np.zeros((2, 128), dtype=_BF16)
    for j in range(6):
        indf[j, 64 * j : 64 * (j + 1)] = 1
    for j in range(2):
        indg[j, 64 * j : 64 * (j + 1)] = 1
    return indf, indg


def _pack_x_core(xc, t_steps):
    """xc [BL, T, D] f32 -> [nblk, 128, XBLK, BL] bf16 (partition = d)."""
    nblk = (t_steps + XBLK - 1) // XBLK
    xt = xc.transpose(1, 2, 0)  # [T, D, BL]
    xt = xt.reshape(nblk, XBLK, D, BL).transpose(0, 2, 1, 3)  # [nblk, D, XBLK, BL]
    return np.ascontiguousarray(xt.astype(_BF16))


TRACE = False  # set by test harness to capture a HW profile
LAST_EXEC_NS = None


def kernel(x, W1, b1, W2, b2, Wout, bout):
    global LAST_EXEC_NS
    from concourse.bass_utils import run_bass_kernel_spmd

    x = np.asarray(x)
    W1 = np.asarray(W1, dtype=np.float32)
    b1 = np.asarray(b1, dtype=np.float32)
    W2 = np.asarray(W2, dtype=np.float32)
    b2 = np.asarray(b2, dtype=np.float32)
    Wout = np.asarray(Wout, dtype=np.float32)
    bout = np.asarray(bout, dtype=np.float32)
    t_steps = x.shape[1]

    with_b1 = bool(np.any(b1))
    with_b2 = bool(np.any(b2))
    nc = _get_nc(t_steps, with_b1, with_b2)

    w1h = _pack_w(W1, 3)
    w2h = _pack_w(W2, 4)
    base = {"w1": w1h, "w2": w2h}
    if with_b1:
        base["b1f"], base["b1g"] = _pack_bias(b1)
    if with_b2:
        base["b2f"], base["b2g"] = _pack_bias(b2)
    if with_b1 or with_b2:
        base["indf"], base["indg"] = _make_indicators()

    in_maps = []
    for i in range(NCORES):
        m = dict(base)
        m["x"] = _pack_x_core(x[i * BL : (i + 1) * BL].astype(np.float32), t_steps)
        in_maps.append(m)

    res = run_bass_kernel_spmd(nc, in_maps, list(range(NCORES)), trace=TRACE)
    LAST_EXEC_NS = res.exec_time_ns

    h2 = np.concatenate(
        [
            res.results[i]["y"].reshape(128, 2, 64).transpose(2, 1, 0).reshape(64, 256)
            for i in range(NCORES)
        ],
        axis=0,
    )
    return (h2.astype(np.float32) @ Wout + bout).astype(np.float32)

